# revision 23
# baseline (speedup 1.0000x reference)
"""Bass/Tile Trainium2 kernel for nn_BcosGCNLayer (b-cos linear layer, B=2).

reference:
    lin  = z @ W.T
    cos  = normalize(z) @ normalize(W).T
    out  = lin * |cos|**(B-1) = lin * |cos|          (B = 2)

Key identity used here: with
    W~ = W * ||w_row||^(-1/2)   (row-wise)
    P  = z @ W~.T = lin / sqrt(||w||)        [per column o]
we get  P * |P| * (1/||z_n||) = lin * |lin| / (||z||*||w||) = lin * |cos| = out.
One GEMM; the epilogue is A = |P| * inv_zn (one ACT op — inv_zn is
per-partition in the [n, o] output tile layout, so it rides the
activation's scale operand) followed by out = P * A (one DVE op).

Sharding: data-parallel on rows across 8 cores (12500 rows/core, padded to
12544 = 98*128); weight replicated. Loads ride the HWDGE (sync) queue,
stores the SWDGE (gpsimd) queue so a store waiting on compute never blocks
a load. The z tiles are transposed on the PE (fp32), the GEMM runs in
fp32r (full PE rate at N=512).
"""

import numpy as np

import concourse.bacc as bacc
import concourse.bass as bass
import concourse.mybir as mybir
import concourse.tile as tile
from concourse import masks

P = 128
D = 512
KB = D // P  # 4 blocks of 128 along the feature dim
N_CORES = 8
TOTAL_ROWS = 100000
ROWS_PER_CORE_RAW = TOTAL_ROWS // N_CORES  # 12500
TILES_PER_CORE = -(-ROWS_PER_CORE_RAW // P)  # 98
ROWS_PER_CORE = TILES_PER_CORE * P  # 12544

F32 = mybir.dt.float32
F32R = mybir.dt.float32r
ACT = mybir.ActivationFunctionType


STORE_ENGINE = "gpsimd"
ABS_ON_DVE_EVERY = 0  # every Nth tile abs on DVE instead of ACT (0=off)
USE_SEQ_CODEGEN = False


def build_kernel(rows: int = ROWS_PER_CORE, stats_batch: int = 4) -> bass.Bass:
    """Build the per-core Bass program: z [rows, 512] -> out [rows, 512]."""
    assert rows % P == 0
    n_tiles = rows // P

    nc = bacc.Bacc()
    z_dram = nc.dram_tensor("z", [rows, D], F32, kind="ExternalInput")
    w_dram = nc.dram_tensor("w", [D, D], F32, kind="ExternalInput")
    out_dram = nc.dram_tensor("out", [rows, D], F32, kind="ExternalOutput")

    with tile.TileContext(nc) as tc:
        with (
            tc.tile_pool(name="consts", bufs=1) as consts,
            tc.tile_pool(name="wprep", bufs=1) as wprep,
            tc.tile_pool(name="zin", bufs=10) as zin_pool,
            tc.tile_pool(name="scratch", bufs=1) as scratch_pool,
            tc.tile_pool(name="stats", bufs=8) as stats_pool,
            tc.tile_pool(name="zt", bufs=10) as zt_pool,
            tc.tile_pool(name="absb", bufs=6) as abs_pool,
            tc.tile_pool(name="outb", bufs=6) as out_pool,
            tc.tile_pool(name="psum_t", bufs=4, space=bass.MemorySpace.PSUM) as pt_pool,
            tc.tile_pool(name="psum_o", bufs=4, space=bass.MemorySpace.PSUM) as po_pool,
        ):
            ident = consts.tile([P, P], F32)
            masks.make_identity(nc, ident[:])
            # PE warmup: absorbs the identity-producer wait into a single
            # instruction so later PE ops carry at most one foreign wait
            # (the per-instruction sync-wait slots are scarce on PE).
            warm = pt_pool.tile([P, P], F32, name="psum_t")
            nc.tensor.transpose(warm[:], ident[:], ident[:])

            # persistent W~T tiles: [i-block k][i=128, o=512]
            wT = wprep.tile([P, KB, D], F32R)

            # ---------- emission helpers ----------------------------------
            def batch_front(c0, nb):
                """loads + row-stats accumulation + transposes + copyback.
                Loads are paired: one 512KB DMA covers two 128-row tiles."""
                assert nb % 2 == 0
                ssq = stats_pool.tile([P, stats_batch], F32, name="ssq")
                ztiles = []
                for jj in range(0, nb, 2):
                    t0 = c0 + jj
                    zpair = zin_pool.tile([P, 2, D], F32, name="z_nat")
                    nc.sync.dma_start(
                        zpair[:],
                        z_dram[t0 * P : (t0 + 2) * P, :].rearrange(
                            "(a p) d -> p a d", p=P
                        ),
                    )
                    for h in range(2):
                        j = jj + h
                        zt_nat = zpair[:, h, :]
                        zsq_scr = scratch_pool.tile([P, D], F32, name="zsq_scr")
                        nc.scalar.activation(
                            zsq_scr[:], zt_nat, ACT.Square,
                            accum_out=ssq[:, j : j + 1],
                        )
                        ptz = pt_pool.tile([P, KB, P], F32, name="psum_t")
                        for k in range(KB):
                            nc.tensor.transpose(
                                ptz[:, k, :], zt_nat[:, k * P : (k + 1) * P], ident[:]
                            )
                        ztile = zt_pool.tile([P, KB, P], F32R, name="ztile")
                        nc.vector.tensor_copy(
                            ztile[:].rearrange("p a b -> p (a b)"),
                            ptz[:].rearrange("p a b -> p (a b)"),
                        )
                        ztiles.append(ztile)
                return ssq, ztiles

            def batch_back(c0, nb, ssq, ztiles):
                """GEMMs + inv-norm + epilogue + stores."""
                pos = []
                for j in range(nb):
                    po = po_pool.tile([P, D], F32, name="psum_o")
                    for k in range(KB):
                        nc.tensor.matmul(
                            po[:],
                            ztiles[j][:, k, :],
                            wT[:, k, :],
                            start=(k == 0),
                            stop=(k == KB - 1),
                        )
                    pos.append(po)
                znrm = stats_pool.tile([P, stats_batch], F32, name="znrm")
                nc.scalar.activation(znrm[:, :nb], ssq[:, :nb], ACT.Sqrt)
                zscale_d = stats_pool.tile([P, stats_batch], F32, name="zscale_d")
                nc.vector.reciprocal(zscale_d[:, :nb], znrm[:, :nb])
                # bounce through ACT so the abs op's scale dep is same-engine
                zscale = stats_pool.tile([P, stats_batch], F32, name="zscale")
                nc.scalar.copy(zscale[:, :nb], zscale_d[:, :nb])
                ot2 = None
                for j in range(nb):
                    t = c0 + j
                    po = pos[j]
                    ab = abs_pool.tile([P, D], F32, name="ab")
                    if ABS_ON_DVE_EVERY and t % ABS_ON_DVE_EVERY == ABS_ON_DVE_EVERY - 1:
                        # balance: fused |P|*inv_zn on DVE instead of ACT
                        nc.vector.tensor_scalar(
                            ab[:], po[:], 0.0, zscale[:, j : j + 1],
                            mybir.AluOpType.abs_max, mybir.AluOpType.mult,
                        )
                    else:
                        nc.scalar.activation(
                            ab[:], po[:], ACT.Abs, scale=zscale[:, j : j + 1]
                        )
                    if j % 2 == 0:
                        ot2 = out_pool.tile([P, 2, D], F32, name="ot")
                    nc.vector.tensor_mul(ot2[:, j % 2, :], po[:], ab[:])
                    if j % 2 == 1:
                        # paired 512KB store on the SWDGE queue: never blocks loads
                        getattr(nc, STORE_ENGINE).dma_start(
                            out_dram[(t - 1) * P : (t + 1) * P, :].rearrange(
                                "(a p) d -> p a d", p=P
                            ),
                            ot2[:],
                        )

            def w_prep_stats():
                """W loads + norm-scale chain (no PE work): runs while the
                first z tiles stream in."""
                w_nat = wprep.tile([P, KB, D], F32)
                nc.sync.dma_start(
                    w_nat[:], w_dram[:].rearrange("(b p) d -> p b d", p=P)
                )
                wsq_scratch = wprep.tile([P, D], F32)
                wssq = wprep.tile([P, KB], F32)
                for b in range(KB):
                    nc.scalar.activation(
                        wsq_scratch[:], w_nat[:, b, :], ACT.Square,
                        accum_out=wssq[:, b : b + 1],
                    )
                wnrm = wprep.tile([P, KB], F32)
                nc.scalar.activation(wnrm[:], wssq[:], ACT.Sqrt)  # ||w||
                wnrm2 = wprep.tile([P, KB], F32)
                nc.scalar.activation(wnrm2[:], wnrm[:], ACT.Sqrt)  # ||w||^(1/2)
                wscale = wprep.tile([P, KB], F32)
                nc.vector.reciprocal(wscale[:], wnrm2[:])  # ||w||^(-1/2)
                # DVE-sourced copies of both W-matmul operands so the W PE
                # matmuls wait on a single engine's semaphore.
                w_nat2 = wprep.tile([P, KB, D], F32)
                nc.vector.tensor_copy(
                    w_nat2[:].rearrange("p a b -> p (a b)"),
                    w_nat[:].rearrange("p a b -> p (a b)"),
                )
                # diag(s_w) per o-block, for the fused scale+transpose matmul
                dsw = wprep.tile([P, KB, P], F32)
                for b in range(KB):
                    nc.vector.tensor_scalar_mul(
                        dsw[:, b, :], ident[:], wscale[:, b : b + 1]
                    )
                return w_nat2, dsw

            def w_prep_pe(w_nat, dsw):
                """One fused scale+transpose matmul per (o-block, i-block):
                W.T @ diag(s_w) = (s_w * W).T"""
                for k in range(KB):
                    pw = pt_pool.tile([P, KB, P], F32, name="psum_t")
                    for b in range(KB):
                        nc.tensor.matmul(
                            pw[:, b, :],
                            w_nat[:, b, k * P : (k + 1) * P],
                            dsw[:, b, :],
                        )
                    nc.vector.tensor_copy(
                        wT[:, k, :], pw[:].rearrange("p a b -> p (a b)")
                    )

            # ---------- emission order ------------------------------------
            # W chain (DMA+ACT+Pool) starts immediately; three batches of z
            # front-work keep PE/DMA busy while it completes; W PE matmuls
            # then slot in, batch-0 GEMMs follow.
            batches = [
                (c0, min(stats_batch, n_tiles - c0))
                for c0 in range(0, n_tiles, stats_batch)
            ]
            LOOKAHEAD = 3
            w_nat, dsw = w_prep_stats()
            fronts = {}
            for i in range(min(LOOKAHEAD, len(batches))):
                c0, nb = batches[i]
                fronts[i] = batch_front(c0, nb)
            w_prep_pe(w_nat, dsw)
            for i, (c0, nb) in enumerate(batches):
                if i + LOOKAHEAD < len(batches):
                    nc0, nnb = batches[i + LOOKAHEAD]
                    fronts[i + LOOKAHEAD] = batch_front(nc0, nnb)
                ssq, ztiles = fronts.pop(i)
                batch_back(c0, nb, ssq, ztiles)

    nc.compile()
    return nc


_NC_CACHE: dict = {}


def _get_nc(rows: int) -> bass.Bass:
    if rows not in _NC_CACHE:
        _NC_CACHE[rows] = build_kernel(rows)
    return _NC_CACHE[rows]


def kernel(z: np.ndarray, weight: np.ndarray) -> np.ndarray:
    """Full-input entry point: z [100000, 512] f32, weight [512, 512] f32."""
    from concourse.bass_utils import run_bass_kernel_spmd

    z = np.ascontiguousarray(z, dtype=np.float32)
    weight = np.ascontiguousarray(weight, dtype=np.float32)
    n_rows = z.shape[0]
    per_core = -(-n_rows // N_CORES)
    per_core_pad = -(-per_core // P) * P

    nc = _get_nc(per_core_pad)

    in_maps = []
    for c in range(N_CORES):
        lo = c * per_core
        hi = min(n_rows, (c + 1) * per_core)
        shard = np.zeros((per_core_pad, D), dtype=np.float32)
        shard[: hi - lo] = z[lo:hi]
        in_maps.append({"z": shard, "w": weight})

    res = run_bass_kernel_spmd(nc, in_maps, core_ids=list(range(N_CORES)))
    out = np.empty((n_rows, D), dtype=np.float32)
    for c in range(N_CORES):
        lo = c * per_core
        hi = min(n_rows, (c + 1) * per_core)
        out[lo:hi] = res.results[c]["out"][: hi - lo]
    return out


# revision 29
# speedup vs baseline: 337.3904x; 337.3904x over previous
"""Bass/Tile Trainium2 kernel for nn_BcosGCNLayer (b-cos linear layer, B=2).

reference:
    lin  = z @ W.T
    cos  = normalize(z) @ normalize(W).T
    out  = lin * |cos|**(B-1) = lin * |cos|          (B = 2)

Key identity used here: with
    W~ = W * ||w_row||^(-1/2)   (row-wise)
    P  = z @ W~.T = lin / sqrt(||w||)        [per column o]
we get  P * |P| * (1/||z_n||) = lin * |lin| / (||z||*||w||) = lin * |cos| = out.
One GEMM; the epilogue is A = |P| * inv_zn (one ACT op — inv_zn is
per-partition in the [n, o] output tile layout, so it rides the
activation's scale operand) followed by out = P * A (one DVE op).

Sharding: data-parallel on rows across 8 cores (12500 rows/core, padded to
12544 = 98*128); weight replicated. Loads ride the HWDGE (sync) queue,
stores the SWDGE (gpsimd) queue so a store waiting on compute never blocks
a load. The z tiles are transposed on the PE (fp32), the GEMM runs in
fp32r (full PE rate at N=512).
"""

import numpy as np

import concourse.bacc as bacc
import concourse.bass as bass
import concourse.mybir as mybir
import concourse.tile as tile
from concourse import masks

P = 128
D = 512
KB = D // P  # 4 blocks of 128 along the feature dim
N_CORES = 8
TOTAL_ROWS = 100000
ROWS_PER_CORE_RAW = TOTAL_ROWS // N_CORES  # 12500
TILES_PER_CORE = -(-ROWS_PER_CORE_RAW // P)  # 98
ROWS_PER_CORE = TILES_PER_CORE * P  # 12544

F32 = mybir.dt.float32
F32R = mybir.dt.float32r
ACT = mybir.ActivationFunctionType


STORE_ENGINE = "gpsimd"
ABS_ON_DVE_EVERY = 0  # every Nth tile abs on DVE instead of ACT (0=off)
USE_SEQ_CODEGEN = False


def build_kernel(
    rows: int = ROWS_PER_CORE,
    stats_batch: int = 4,
    repeat: int = 1,
    alias_rows: int = 0,
    hw_loop: int = 0,
) -> bass.Bass:
    """Build the per-core Bass program: z [rows, 512] -> out [rows, 512].

    alias_rows (bench only): allocate the DRAM tensors with `alias_rows`
    rows and wrap all row addressing mod alias_rows. DMA/compute work per
    iteration is unchanged, but host<->device shipping shrinks to ~nothing,
    letting wall-clock expose true device time.
    """
    assert rows % P == 0
    n_tiles = rows // P
    dram_rows = alias_rows or rows

    nc = bacc.Bacc()
    z_dram = nc.dram_tensor("z", [dram_rows, D], F32, kind="ExternalInput")
    w_dram = nc.dram_tensor("w", [D, D], F32, kind="ExternalInput")
    out_dram = nc.dram_tensor("out", [dram_rows, D], F32, kind="ExternalOutput")

    def rowslice(dram, t0, ntile):
        r = (t0 * P) % dram_rows
        return dram[r : r + ntile * P, :]

    with tile.TileContext(nc) as tc:
        with (
            tc.tile_pool(name="consts", bufs=1) as consts,
            tc.tile_pool(name="wprep", bufs=1) as wprep,
            tc.tile_pool(name="zin", bufs=10) as zin_pool,
            tc.tile_pool(name="scratch", bufs=1) as scratch_pool,
            tc.tile_pool(name="stats", bufs=8) as stats_pool,
            tc.tile_pool(name="zt", bufs=10) as zt_pool,
            tc.tile_pool(name="absb", bufs=6) as abs_pool,
            tc.tile_pool(name="outb", bufs=6) as out_pool,
            tc.tile_pool(name="psum_t", bufs=4, space=bass.MemorySpace.PSUM) as pt_pool,
            tc.tile_pool(name="psum_o", bufs=4, space=bass.MemorySpace.PSUM) as po_pool,
        ):
            ident = consts.tile([P, P], F32)
            masks.make_identity(nc, ident[:])
            # PE warmup: absorbs the identity-producer wait into a single
            # instruction so later PE ops carry at most one foreign wait
            # (the per-instruction sync-wait slots are scarce on PE).
            warm = pt_pool.tile([P, P], F32, name="psum_t")
            nc.tensor.transpose(warm[:], ident[:], ident[:])

            # persistent W~T tiles: [i-block k][i=128, o=512]
            wT = wprep.tile([P, KB, D], F32R)

            # ---------- emission helpers ----------------------------------
            def batch_front(c0, nb):
                """loads + row-stats accumulation + transposes + copyback.
                Loads are paired: one 512KB DMA covers two 128-row tiles."""
                assert nb % 2 == 0
                ssq = stats_pool.tile([P, stats_batch], F32, name="ssq")
                ztiles = []
                for jj in range(0, nb, 2):
                    t0 = c0 + jj
                    zpair = zin_pool.tile([P, 2, D], F32, name="z_nat")
                    nc.sync.dma_start(
                        zpair[:],
                        rowslice(z_dram, t0, 2).rearrange("(a p) d -> p a d", p=P),
                    )
                    for h in range(2):
                        j = jj + h
                        zt_nat = zpair[:, h, :]
                        zsq_scr = scratch_pool.tile([P, D], F32, name="zsq_scr")
                        nc.scalar.activation(
                            zsq_scr[:], zt_nat, ACT.Square,
                            accum_out=ssq[:, j : j + 1],
                        )
                        ptz = pt_pool.tile([P, KB, P], F32, name="psum_t")
                        for k in range(KB):
                            nc.tensor.transpose(
                                ptz[:, k, :], zt_nat[:, k * P : (k + 1) * P], ident[:]
                            )
                        ztile = zt_pool.tile([P, KB, P], F32R, name="ztile")
                        nc.vector.tensor_copy(
                            ztile[:].rearrange("p a b -> p (a b)"),
                            ptz[:].rearrange("p a b -> p (a b)"),
                        )
                        ztiles.append(ztile)
                return ssq, ztiles

            def batch_back(c0, nb, ssq, ztiles):
                """GEMMs + inv-norm + epilogue + stores."""
                pos = []
                for j in range(nb):
                    po = po_pool.tile([P, D], F32, name="psum_o")
                    for k in range(KB):
                        nc.tensor.matmul(
                            po[:],
                            ztiles[j][:, k, :],
                            wT[:, k, :],
                            start=(k == 0),
                            stop=(k == KB - 1),
                        )
                    pos.append(po)
                znrm = stats_pool.tile([P, stats_batch], F32, name="znrm")
                nc.scalar.activation(znrm[:, :nb], ssq[:, :nb], ACT.Sqrt)
                zscale_d = stats_pool.tile([P, stats_batch], F32, name="zscale_d")
                nc.vector.reciprocal(zscale_d[:, :nb], znrm[:, :nb])
                # bounce through ACT so the abs op's scale dep is same-engine
                zscale = stats_pool.tile([P, stats_batch], F32, name="zscale")
                nc.scalar.copy(zscale[:, :nb], zscale_d[:, :nb])
                ot2 = None
                for j in range(nb):
                    t = c0 + j
                    po = pos[j]
                    ab = abs_pool.tile([P, D], F32, name="ab")
                    if ABS_ON_DVE_EVERY and t % ABS_ON_DVE_EVERY == ABS_ON_DVE_EVERY - 1:
                        # balance: fused |P|*inv_zn on DVE instead of ACT
                        nc.vector.tensor_scalar(
                            ab[:], po[:], 0.0, zscale[:, j : j + 1],
                            mybir.AluOpType.abs_max, mybir.AluOpType.mult,
                        )
                    else:
                        nc.scalar.activation(
                            ab[:], po[:], ACT.Abs, scale=zscale[:, j : j + 1]
                        )
                    if j % 2 == 0:
                        ot2 = out_pool.tile([P, 2, D], F32, name="ot")
                    nc.vector.tensor_mul(ot2[:, j % 2, :], po[:], ab[:])
                    if j % 2 == 1:
                        # paired 512KB store on the SWDGE queue: never blocks loads
                        getattr(nc, STORE_ENGINE).dma_start(
                            rowslice(out_dram, t - 1, 2).rearrange(
                                "(a p) d -> p a d", p=P
                            ),
                            ot2[:],
                        )

            def w_prep_stats():
                """W loads + norm-scale chain (no PE work): runs while the
                first z tiles stream in."""
                w_nat = wprep.tile([P, KB, D], F32)
                nc.sync.dma_start(
                    w_nat[:], w_dram[:].rearrange("(b p) d -> p b d", p=P)
                )
                wsq_scratch = wprep.tile([P, D], F32)
                wssq = wprep.tile([P, KB], F32)
                for b in range(KB):
                    nc.scalar.activation(
                        wsq_scratch[:], w_nat[:, b, :], ACT.Square,
                        accum_out=wssq[:, b : b + 1],
                    )
                wnrm = wprep.tile([P, KB], F32)
                nc.scalar.activation(wnrm[:], wssq[:], ACT.Sqrt)  # ||w||
                wnrm2 = wprep.tile([P, KB], F32)
                nc.scalar.activation(wnrm2[:], wnrm[:], ACT.Sqrt)  # ||w||^(1/2)
                wscale = wprep.tile([P, KB], F32)
                nc.vector.reciprocal(wscale[:], wnrm2[:])  # ||w||^(-1/2)
                # DVE-sourced copies of both W-matmul operands so the W PE
                # matmuls wait on a single engine's semaphore.
                w_nat2 = wprep.tile([P, KB, D], F32)
                nc.vector.tensor_copy(
                    w_nat2[:].rearrange("p a b -> p (a b)"),
                    w_nat[:].rearrange("p a b -> p (a b)"),
                )
                # diag(s_w) per o-block, for the fused scale+transpose matmul
                dsw = wprep.tile([P, KB, P], F32)
                for b in range(KB):
                    nc.vector.tensor_scalar_mul(
                        dsw[:, b, :], ident[:], wscale[:, b : b + 1]
                    )
                return w_nat2, dsw

            def w_prep_pe(w_nat, dsw):
                """One fused scale+transpose matmul per (o-block, i-block):
                W.T @ diag(s_w) = (s_w * W).T"""
                for k in range(KB):
                    pw = pt_pool.tile([P, KB, P], F32, name="psum_t")
                    for b in range(KB):
                        nc.tensor.matmul(
                            pw[:, b, :],
                            w_nat[:, b, k * P : (k + 1) * P],
                            dsw[:, b, :],
                        )
                    nc.vector.tensor_copy(
                        wT[:, k, :], pw[:].rearrange("p a b -> p (a b)")
                    )

            # ---------- emission order ------------------------------------
            # W chain (DMA+ACT+Pool) starts immediately; three batches of z
            # front-work keep PE/DMA busy while it completes; W PE matmuls
            # then slot in, batch-0 GEMMs follow.
            batches = [
                (c0, min(stats_batch, n_tiles - c0))
                for c0 in range(0, n_tiles, stats_batch)
            ]
            LOOKAHEAD = 3

            def emit_passes(n_passes, prime=True):
                """Self-contained pipelined emission of n_passes full row
                loops (software-pipelined across pass boundaries)."""
                all_batches = batches * n_passes
                fronts = {}
                for i in range(min(LOOKAHEAD, len(all_batches))):
                    c0, nb = all_batches[i]
                    fronts[i] = batch_front(c0, nb)
                yield  # caller may interleave W-prep PE work here
                for i in range(len(all_batches)):
                    c0, nb = all_batches[i]
                    if i + LOOKAHEAD < len(all_batches):
                        nc0, nnb = all_batches[i + LOOKAHEAD]
                        fronts[i + LOOKAHEAD] = batch_front(nc0, nnb)
                    ssq, ztiles = fronts.pop(i)
                    batch_back(c0, nb, ssq, ztiles)

            w_nat, dsw = w_prep_stats()
            if hw_loop:
                # bench mode: W-prep fully first, then a hardware loop whose
                # body is `repeat` self-contained passes.
                w_prep_pe(w_nat, dsw)
                with tc.For_i(
                    0, hw_loop, 1,
                    hint_engines=(mybir.EngineType.PE, mybir.EngineType.Activation,
                                  mybir.EngineType.DVE, mybir.EngineType.SP,
                                  mybir.EngineType.Pool),
                ):
                    for _ in emit_passes(repeat):
                        pass
            else:
                gen = emit_passes(repeat)
                next(gen)          # primes LOOKAHEAD fronts
                w_prep_pe(w_nat, dsw)
                for _ in gen:      # drains the rest of the emission
                    pass

    nc.compile()
    return nc


_NC_CACHE: dict = {}


def _get_nc(rows: int) -> bass.Bass:
    if rows not in _NC_CACHE:
        _NC_CACHE[rows] = build_kernel(rows)
    return _NC_CACHE[rows]


def kernel(z: np.ndarray, weight: np.ndarray) -> np.ndarray:
    """Full-input entry point: z [100000, 512] f32, weight [512, 512] f32."""
    from concourse.bass_utils import run_bass_kernel_spmd

    z = np.ascontiguousarray(z, dtype=np.float32)
    weight = np.ascontiguousarray(weight, dtype=np.float32)
    n_rows = z.shape[0]
    per_core = -(-n_rows // N_CORES)
    per_core_pad = -(-per_core // P) * P

    nc = _get_nc(per_core_pad)

    in_maps = []
    for c in range(N_CORES):
        lo = c * per_core
        hi = min(n_rows, (c + 1) * per_core)
        shard = np.zeros((per_core_pad, D), dtype=np.float32)
        shard[: hi - lo] = z[lo:hi]
        in_maps.append({"z": shard, "w": weight})

    res = run_bass_kernel_spmd(nc, in_maps, core_ids=list(range(N_CORES)))
    out = np.empty((n_rows, D), dtype=np.float32)
    for c in range(N_CORES):
        lo = c * per_core
        hi = min(n_rows, (c + 1) * per_core)
        out[lo:hi] = res.results[c]["out"][: hi - lo]
    return out
